# revision 9
# baseline (speedup 1.0000x reference)
"""BiLSTM-CRF Trainium2 kernel.

Sharding: 8 cores = 2 directions x 4 batch-groups of 8 examples.
Each core runs the same Bass program on different data:
  phase 1: P = X @ Wih.T + b      (parallel GEMM over all tokens -> DRAM)
  phase 2: LSTM scan over S steps (PE: h.T stationary, Whh.T streaming;
           ACT: gate nonlinearities; DVE: cell update; PE-transpose h)
  phase 3: feats_half = H_seq @ W_out_half.T
Host: embedding gather, time-reversal for the backward direction,
summing the two feature halves + b_out, Viterbi DP + backtrace.
"""

import numpy as np
from contextlib import ExitStack

import concourse.bass as bass
from concourse import bacc
import concourse.mybir as mybir
from concourse import tile
from concourse.bass_utils import run_bass_kernel_spmd

F32 = mybir.dt.float32
AF = mybir.ActivationFunctionType

B, S, E, H, T = 32, 512, 512, 512, 16
G4 = 4 * H          # 2048 gates
NCORES = 8
NGRP = 4            # batch groups
BL = B // NGRP      # 8 examples per core


def build_program(nc, s_len=S, bl=BL):
    toks = bl * s_len
    assert s_len % 128 == 0
    xt = nc.declare_dram_parameter("xt", [E, toks], F32, isOutput=False)
    wih = nc.declare_dram_parameter("wih", [E, G4], F32, isOutput=False)
    whh = nc.declare_dram_parameter("whh", [H, G4], F32, isOutput=False)
    bb = nc.declare_dram_parameter("bb", [128, G4], F32, isOutput=False)
    wo = nc.declare_dram_parameter("wo", [H, T], F32, isOutput=False)
    ident = nc.declare_dram_parameter("ident", [bl, bl], F32, isOutput=False)
    feats = nc.declare_dram_parameter("feats", [bl, T, s_len], F32, isOutput=True)
    pd = nc.dram_tensor("pscr", [bl, s_len, G4], F32)

    KE = E // 128
    KH = H // 128
    NT = G4 // 512
    MT = toks // 128

    with tile.TileContext(nc) as tc, ExitStack() as ctx:
        wpool = ctx.enter_context(tc.tile_pool(name="persist", bufs=1))
        whh_sb = wpool.tile([128, KH, G4], F32, tag="whh")
        nc.sync.dma_start(whh_sb[:], whh.rearrange("(k p) n -> p k n", p=128))
        wo_sb = wpool.tile([128, KH, T], F32, tag="wo")
        nc.sync.dma_start(wo_sb[:], wo.rearrange("(k p) n -> p k n", p=128))
        id_sb = wpool.tile([bl, bl], F32, tag="id")
        nc.sync.dma_start(id_sb[:], ident[:])
        bb_sb = wpool.tile([128, G4], F32, tag="bb")
        nc.sync.dma_start(bb_sb[:], bb[:])
        # h.T history: [p, k, b, s+1]; slot 0 is h_init = 0, step t writes slot t+1
        ht = wpool.tile([128, KH, bl, s_len], F32, tag="ht")  # h.T history (h_0..h_{S-1})
        htc = wpool.tile([128, KH, bl], F32, tag="htc")  # current h.T (static addr for LDW)
        c_sb = wpool.tile([bl, H], F32, tag="c")
        nc.gpsimd.memset(htc[:], 0.0)
        nc.gpsimd.memset(c_sb[:], 0.0)

        # ---- phase 1: P = X @ Wih.T + b over 128-token tiles ----
        with tc.tile_pool(name="xtl", bufs=3) as xp, \
             tc.tile_pool(name="p1ps", bufs=4, space="PSUM") as pp, \
             tc.tile_pool(name="wihp", bufs=1) as wihp, \
             tc.tile_pool(name="pout", bufs=4) as pop:
            wih_sb = wihp.tile([128, KE, G4], F32)
            nc.sync.dma_start(wih_sb[:], wih.rearrange("(k p) n -> p k n", p=128))
            xtr = xt.rearrange("(k p) n -> p k n", p=128)
            for m in range(MT):
                xt_sb = xp.tile([128, KE, 128], F32)
                nc.sync.dma_start(xt_sb[:], xtr[:, :, m * 128:(m + 1) * 128])
                bidx, s0 = divmod(m * 128, s_len)
                for n in range(NT):
                    ps = pp.tile([128, 512], F32)
                    for k in range(KE):
                        nc.tensor.matmul(
                            ps[:], xt_sb[:, k, :],
                            wih_sb[:, k, n * 512:(n + 1) * 512],
                            start=(k == 0), stop=(k == KE - 1))
                    po = pop.tile([128, 512], F32)
                    nc.vector.tensor_add(po[:], ps[:], bb_sb[:, n * 512:(n + 1) * 512])
                    nc.sync.dma_start(pd[bidx, s0:s0 + 128, n * 512:(n + 1) * 512], po[:])

        # ---- phase 2: sequential scan ----
        with tc.tile_pool(name="ptl", bufs=2) as ptp, \
             tc.tile_pool(name="gsb", bufs=2) as gp, \
             tc.tile_pool(name="gps", bufs=4, space="PSUM") as gpsp, \
             tc.tile_pool(name="tps", bufs=2, space="PSUM") as tpsp:
            with tc.For_i(0, s_len, 1) as t:
                pt_sb = ptp.tile([bl, 1, G4], F32)
                nc.sync.dma_start(pt_sb[:], pd[:, bass.ds(t, 1), :])
                a = {}
                for n, (nm, fn) in enumerate((("i", AF.Sigmoid), ("f", AF.Sigmoid),
                                              ("g", AF.Tanh), ("o", AF.Sigmoid))):
                    ps = gpsp.tile([bl, 512], F32, tag="gpsum")
                    for k in range(KH):
                        nc.tensor.matmul(
                            ps[:], htc[:, k, :],
                            whh_sb[:, k, n * 512:(n + 1) * 512],
                            start=(k == 0), stop=(k == KH - 1))
                    gsb = gp.tile([bl, 512], F32, tag="g" + nm)
                    nc.vector.tensor_add(gsb[:], ps[:], pt_sb[:, 0, n * 512:(n + 1) * 512])
                    asb = gp.tile([bl, 512], F32, tag="a" + nm)
                    nc.scalar.activation(asb[:], gsb[:], fn)
                    a[nm] = asb
                t1 = gp.tile([bl, 512], F32, tag="t1")
                nc.vector.tensor_mul(t1[:], a["i"][:], a["g"][:])
                nc.vector.tensor_mul(c_sb[:], c_sb[:], a["f"][:])
                nc.vector.tensor_add(c_sb[:], c_sb[:], t1[:])
                tch = gp.tile([bl, 512], F32, tag="tch")
                nc.scalar.activation(tch[:], c_sb[:], AF.Tanh)
                h_sb = gp.tile([bl, 512], F32, tag="h")
                nc.vector.tensor_mul(h_sb[:], a["o"][:], tch[:])
                tp = tpsp.tile([128, KH, bl, 1], F32, tag="tpsum")
                for k in range(KH):
                    nc.tensor.transpose(tp[:, k, :, 0], h_sb[:, k * 128:(k + 1) * 128], id_sb[:])
                nc.vector.tensor_copy(htc[:], tp[:, :, :, 0])
                nc.scalar.copy(ht[:, :, :, bass.ds(t, 1)], tp[:])

        # ---- phase 3: feats_half.T = WoT.T @ H.T ----
        with tc.tile_pool(name="f3", bufs=2) as f3p, \
             tc.tile_pool(name="f3ps", bufs=2, space="PSUM") as f3ps:
            for bi in range(bl):
                ps = f3ps.tile([T, s_len], F32)
                for k in range(KH):
                    nc.tensor.matmul(ps[:], wo_sb[:, k, :], ht[:, k, bi, :],
                                     start=(k == 0), stop=(k == KH - 1))
                fo = f3p.tile([T, s_len], F32)
                nc.vector.tensor_copy(fo[:], ps[:])
                nc.sync.dma_start(feats[bi], fo[:])
    return nc


_NC_CACHE = {}


def _get_nc():
    if "nc" not in _NC_CACHE:
        nc = bacc.Bacc("TRN2")
        build_program(nc)
        nc.finalize()
        _NC_CACHE["nc"] = nc
    return _NC_CACHE["nc"]


def make_in_maps(emb, Wih_f, Whh_f, b_f, Wih_b, Whh_b, b_b, W_out, s_len=S, bl=BL):
    """emb: [B, s_len, E] float32. Returns 8 per-core input maps."""
    in_maps = []
    for c in range(NCORES):
        d, g = divmod(c, NGRP)
        x = emb[g * bl:(g + 1) * bl]
        if d == 1:
            x = x[:, ::-1]
        xtm = np.ascontiguousarray(x.reshape(bl * s_len, E).T).astype(np.float32)
        Wih, Whh, bvec = (Wih_f, Whh_f, b_f) if d == 0 else (Wih_b, Whh_b, b_b)
        wo_half = W_out[:, :H] if d == 0 else W_out[:, H:]
        in_maps.append({
            "xt": xtm,
            "wih": np.ascontiguousarray(np.asarray(Wih, np.float32).T),
            "whh": np.ascontiguousarray(np.asarray(Whh, np.float32).T),
            "bb": np.tile(np.asarray(bvec, np.float32)[None, :], (128, 1)),
            "wo": np.ascontiguousarray(np.asarray(wo_half, np.float32).T),
            "ident": np.eye(bl, dtype=np.float32),
        })
    return in_maps


def assemble_feats(results, b_out, s_len=S, bl=BL):
    feats = np.zeros((NGRP * bl, s_len, T), np.float32)
    for c in range(NCORES):
        d, g = divmod(c, NGRP)
        f = np.transpose(np.asarray(results[c]["feats"]), (0, 2, 1))  # [bl, s, T]
        if d == 1:
            f = f[:, ::-1]
        feats[g * bl:(g + 1) * bl] += f
    feats += np.asarray(b_out, np.float32)[None, None, :]
    return feats


def viterbi(feats, trans, start, stop):
    Bq, Sq, Tq = feats.shape
    v = feats[:, 0] + start[None, :]
    idxs = np.zeros((Sq - 1, Bq, Tq), np.int32)
    for s in range(1, Sq):
        scores = v[:, :, None] + trans[None, :, :]
        idxs[s - 1] = np.argmax(scores, axis=1)
        v = np.max(scores, axis=1) + feats[:, s]
    last = np.argmax(v + stop[None, :], axis=-1).astype(np.int32)
    tags = np.zeros((Bq, Sq), np.int32)
    tags[:, -1] = last
    cur = last
    for s in range(Sq - 2, -1, -1):
        cur = idxs[s][np.arange(Bq), cur].astype(np.int32)
        tags[:, s] = cur
    return tags


def kernel(sentence, embedding, Wih_f, Whh_f, b_f, Wih_b, Whh_b, b_b,
           W_out, b_out, transitions, start_trans, stop_trans):
    sentence = np.asarray(sentence)
    emb = np.asarray(embedding, np.float32)[sentence.astype(np.int64)]  # [B, S, E]
    nc = _get_nc()
    in_maps = make_in_maps(emb, np.asarray(Wih_f), np.asarray(Whh_f), np.asarray(b_f),
                           np.asarray(Wih_b), np.asarray(Whh_b), np.asarray(b_b),
                           np.asarray(W_out))
    res = run_bass_kernel_spmd(nc, in_maps, list(range(NCORES))).results
    feats = assemble_feats(res, np.asarray(b_out))
    return viterbi(feats, np.asarray(transitions, np.float32),
                   np.asarray(start_trans, np.float32),
                   np.asarray(stop_trans, np.float32))


# revision 10
# speedup vs baseline: 3.1916x; 3.1916x over previous
"""BiLSTM-CRF Trainium2 kernel.

Sharding: 8 cores = 2 directions x 4 batch-groups of 8 examples.
Each core runs the same Bass program on different data:
  phase 1: P = X @ Wih.T + b      (parallel GEMM over all tokens -> DRAM)
  phase 2: LSTM scan over S steps (PE: h.T stationary, Whh.T streaming;
           ACT: gate nonlinearities; DVE: cell update; PE-transpose h)
  phase 3: feats_half = H_seq @ W_out_half.T
Host: embedding gather, time-reversal for the backward direction,
summing the two feature halves + b_out, Viterbi DP + backtrace.
"""

import numpy as np
from contextlib import ExitStack

import concourse.bass as bass
from concourse import bacc
import concourse.mybir as mybir
from concourse import tile
from concourse.bass_utils import run_bass_kernel_spmd

F32 = mybir.dt.float32
AF = mybir.ActivationFunctionType

B, S, E, H, T = 32, 512, 512, 512, 16
G4 = 4 * H          # 2048 gates
NCORES = 8
NGRP = 4            # batch groups
BL = B // NGRP      # 8 examples per core


def build_program(nc, s_len=S, bl=BL):
    toks = bl * s_len
    assert s_len % 128 == 0
    xt = nc.declare_dram_parameter("xt", [E, toks], F32, isOutput=False)
    wih = nc.declare_dram_parameter("wih", [E, G4], F32, isOutput=False)
    whh = nc.declare_dram_parameter("whh", [H, G4], F32, isOutput=False)
    bb = nc.declare_dram_parameter("bb", [128, G4], F32, isOutput=False)
    wo = nc.declare_dram_parameter("wo", [H, T], F32, isOutput=False)
    ident = nc.declare_dram_parameter("ident", [bl, bl], F32, isOutput=False)
    feats = nc.declare_dram_parameter("feats", [bl, T, s_len], F32, isOutput=True)
    pd = nc.dram_tensor("pscr", [bl, s_len, G4], F32)

    KE = E // 128
    KH = H // 128
    NT = G4 // 512
    MT = toks // 128

    with tile.TileContext(nc) as tc, ExitStack() as ctx:
        wpool = ctx.enter_context(tc.tile_pool(name="persist", bufs=1))
        whh_sb = wpool.tile([128, KH, G4], F32, tag="whh")
        nc.sync.dma_start(whh_sb[:], whh.rearrange("(k p) n -> p k n", p=128))
        wo_sb = wpool.tile([128, KH, T], F32, tag="wo")
        nc.sync.dma_start(wo_sb[:], wo.rearrange("(k p) n -> p k n", p=128))
        id_sb = wpool.tile([bl, bl], F32, tag="id")
        nc.sync.dma_start(id_sb[:], ident[:])
        bb_sb = wpool.tile([128, G4], F32, tag="bb")
        nc.sync.dma_start(bb_sb[:], bb[:])
        # h.T history: [p, k, b, s+1]; slot 0 is h_init = 0, step t writes slot t+1
        ht = wpool.tile([128, KH, bl, s_len], F32, tag="ht")  # h.T history (h_0..h_{S-1})
        htc = wpool.tile([128, KH, bl], F32, tag="htc")  # current h.T (static addr for LDW)
        c_sb = wpool.tile([bl, H], F32, tag="c")
        nc.gpsimd.memset(htc[:], 0.0)
        nc.gpsimd.memset(c_sb[:], 0.0)

        # ---- phase 1: P = X @ Wih.T + b over 128-token tiles ----
        with tc.tile_pool(name="xtl", bufs=3) as xp, \
             tc.tile_pool(name="p1ps", bufs=4, space="PSUM") as pp, \
             tc.tile_pool(name="wihp", bufs=1) as wihp, \
             tc.tile_pool(name="pout", bufs=4) as pop:
            wih_sb = wihp.tile([128, KE, G4], F32)
            nc.sync.dma_start(wih_sb[:], wih.rearrange("(k p) n -> p k n", p=128))
            xtr = xt.rearrange("(k p) n -> p k n", p=128)
            for m in range(MT):
                xt_sb = xp.tile([128, KE, 128], F32)
                nc.sync.dma_start(xt_sb[:], xtr[:, :, m * 128:(m + 1) * 128])
                bidx, s0 = divmod(m * 128, s_len)
                for n in range(NT):
                    ps = pp.tile([128, 512], F32)
                    for k in range(KE):
                        nc.tensor.matmul(
                            ps[:], xt_sb[:, k, :],
                            wih_sb[:, k, n * 512:(n + 1) * 512],
                            start=(k == 0), stop=(k == KE - 1))
                    po = pop.tile([128, 512], F32)
                    nc.vector.tensor_add(po[:], ps[:], bb_sb[:, n * 512:(n + 1) * 512])
                    nc.sync.dma_start(pd[bidx, s0:s0 + 128, n * 512:(n + 1) * 512], po[:])

        # ---- phase 2: sequential scan (fully static unroll) ----
        with tc.tile_pool(name="ptl", bufs=4) as ptp, \
             tc.tile_pool(name="gsb", bufs=2) as gp, \
             tc.tile_pool(name="gps", bufs=4, space="PSUM") as gpsp, \
             tc.tile_pool(name="tps", bufs=2, space="PSUM") as tpsp:
            for t in range(s_len):
                pt_sb = ptp.tile([bl, 1, G4], F32, tag="pt")
                nc.sync.dma_start(pt_sb[:], pd[:, t:t + 1, :])
                a = {}
                for n, (nm, fn) in enumerate((("i", AF.Sigmoid), ("f", AF.Sigmoid),
                                              ("g", AF.Tanh), ("o", AF.Sigmoid))):
                    ps = gpsp.tile([bl, 512], F32, tag="gpsum")
                    for k in range(KH):
                        lhs = htc[:, k, :] if t == 0 else ht[:, k, :, t - 1]
                        nc.tensor.matmul(
                            ps[:], lhs,
                            whh_sb[:, k, n * 512:(n + 1) * 512],
                            start=(k == 0), stop=(k == KH - 1))
                    gsb = gp.tile([bl, 512], F32, tag="g" + nm)
                    nc.vector.tensor_add(gsb[:], ps[:], pt_sb[:, 0, n * 512:(n + 1) * 512])
                    asb = gp.tile([bl, 512], F32, tag="a" + nm)
                    nc.scalar.activation(asb[:], gsb[:], fn)
                    a[nm] = asb
                t1 = gp.tile([bl, 512], F32, tag="t1")
                nc.vector.tensor_mul(t1[:], a["i"][:], a["g"][:])
                nc.vector.tensor_mul(c_sb[:], c_sb[:], a["f"][:])
                nc.vector.tensor_add(c_sb[:], c_sb[:], t1[:])
                tch = gp.tile([bl, 512], F32, tag="tch")
                nc.scalar.activation(tch[:], c_sb[:], AF.Tanh)
                h_sb = gp.tile([bl, 512], F32, tag="h")
                nc.vector.tensor_mul(h_sb[:], a["o"][:], tch[:])
                tp = tpsp.tile([128, KH, bl, 1], F32, tag="tpsum")
                for k in range(KH):
                    nc.tensor.transpose(tp[:, k, :, 0], h_sb[:, k * 128:(k + 1) * 128], id_sb[:])
                nc.scalar.copy(ht[:, :, :, t:t + 1], tp[:])

        # ---- phase 3: feats_half.T = WoT.T @ H.T ----
        with tc.tile_pool(name="f3", bufs=2) as f3p, \
             tc.tile_pool(name="f3ps", bufs=2, space="PSUM") as f3ps:
            for bi in range(bl):
                ps = f3ps.tile([T, s_len], F32)
                for k in range(KH):
                    nc.tensor.matmul(ps[:], wo_sb[:, k, :], ht[:, k, bi, :],
                                     start=(k == 0), stop=(k == KH - 1))
                fo = f3p.tile([T, s_len], F32)
                nc.vector.tensor_copy(fo[:], ps[:])
                nc.sync.dma_start(feats[bi], fo[:])
    return nc


_NC_CACHE = {}


def _get_nc():
    if "nc" not in _NC_CACHE:
        nc = bacc.Bacc("TRN2")
        build_program(nc)
        nc.finalize()
        _NC_CACHE["nc"] = nc
    return _NC_CACHE["nc"]


def make_in_maps(emb, Wih_f, Whh_f, b_f, Wih_b, Whh_b, b_b, W_out, s_len=S, bl=BL):
    """emb: [B, s_len, E] float32. Returns 8 per-core input maps."""
    in_maps = []
    for c in range(NCORES):
        d, g = divmod(c, NGRP)
        x = emb[g * bl:(g + 1) * bl]
        if d == 1:
            x = x[:, ::-1]
        xtm = np.ascontiguousarray(x.reshape(bl * s_len, E).T).astype(np.float32)
        Wih, Whh, bvec = (Wih_f, Whh_f, b_f) if d == 0 else (Wih_b, Whh_b, b_b)
        wo_half = W_out[:, :H] if d == 0 else W_out[:, H:]
        in_maps.append({
            "xt": xtm,
            "wih": np.ascontiguousarray(np.asarray(Wih, np.float32).T),
            "whh": np.ascontiguousarray(np.asarray(Whh, np.float32).T),
            "bb": np.tile(np.asarray(bvec, np.float32)[None, :], (128, 1)),
            "wo": np.ascontiguousarray(np.asarray(wo_half, np.float32).T),
            "ident": np.eye(bl, dtype=np.float32),
        })
    return in_maps


def assemble_feats(results, b_out, s_len=S, bl=BL):
    feats = np.zeros((NGRP * bl, s_len, T), np.float32)
    for c in range(NCORES):
        d, g = divmod(c, NGRP)
        f = np.transpose(np.asarray(results[c]["feats"]), (0, 2, 1))  # [bl, s, T]
        if d == 1:
            f = f[:, ::-1]
        feats[g * bl:(g + 1) * bl] += f
    feats += np.asarray(b_out, np.float32)[None, None, :]
    return feats


def viterbi(feats, trans, start, stop):
    Bq, Sq, Tq = feats.shape
    v = feats[:, 0] + start[None, :]
    idxs = np.zeros((Sq - 1, Bq, Tq), np.int32)
    for s in range(1, Sq):
        scores = v[:, :, None] + trans[None, :, :]
        idxs[s - 1] = np.argmax(scores, axis=1)
        v = np.max(scores, axis=1) + feats[:, s]
    last = np.argmax(v + stop[None, :], axis=-1).astype(np.int32)
    tags = np.zeros((Bq, Sq), np.int32)
    tags[:, -1] = last
    cur = last
    for s in range(Sq - 2, -1, -1):
        cur = idxs[s][np.arange(Bq), cur].astype(np.int32)
        tags[:, s] = cur
    return tags


def kernel(sentence, embedding, Wih_f, Whh_f, b_f, Wih_b, Whh_b, b_b,
           W_out, b_out, transitions, start_trans, stop_trans):
    sentence = np.asarray(sentence)
    emb = np.asarray(embedding, np.float32)[sentence.astype(np.int64)]  # [B, S, E]
    nc = _get_nc()
    in_maps = make_in_maps(emb, np.asarray(Wih_f), np.asarray(Whh_f), np.asarray(b_f),
                           np.asarray(Wih_b), np.asarray(Whh_b), np.asarray(b_b),
                           np.asarray(W_out))
    res = run_bass_kernel_spmd(nc, in_maps, list(range(NCORES))).results
    feats = assemble_feats(res, np.asarray(b_out))
    return viterbi(feats, np.asarray(transitions, np.float32),
                   np.asarray(start_trans, np.float32),
                   np.asarray(stop_trans, np.float32))
